# revision 38
# baseline (speedup 1.0000x reference)
"""DeltaNet decode step on 8 Trainium2 NeuronCores (tensor-parallel over heads).

Contract: kernel(**inputs) takes the FULL unsharded inputs (numpy arrays,
same keys as the reference setup_inputs()) and returns the FULL output
[1, 4096, 1, 1] float32.

Sharding (8 cores, 16 heads -> 2 heads/core):
  - Wq/Wk rows, q/k conv weights+caches: 512 rows per core
  - Wv rows, v conv weights+caches, Wo columns: 1024 per core
  - state: 2 heads per core
  - output: each core computes a partial [4096] projection; host all-reduces.

Device kernel: memory-bound mat-vec streaming. Weights are host-converted
to single bf16 and packed tile-major (each [128, 16384] SBUF tile is one
contiguous 32 KB row per partition in DRAM). wpool bufs=2 serializes the
weight DMAs into phase order (qk, v, o) so later phases don't steal HBM
bandwidth from earlier ones; the last phases stream in 1 MB quarters so
the final matmuls lag the last byte by only ~2 us.

The post-matvec chain (conv, l2norm, state update, combine) runs in
128-lane column layout. Everything that doesn't depend on v (alpha/beta
broadcast, state matvecs, alpha*ks / alpha*qs) is precomputed during the
v/o streaming phases, leaving a ~4 us serial tail: v fold -> conv tap 3
-> SiLU -> 3 wide DVE ops -> bf16 cast.
"""

import sys
import types

sys.path.insert(0, "/opt/trn_rl_repo")

import numpy as np
import ml_dtypes

import concourse.bass as bass
import concourse.mybir as mybir
import concourse.tile as tile
from concourse import bacc
from concourse.bass_utils import run_bass_kernel_spmd

BF16 = ml_dtypes.bfloat16
F32 = mybir.dt.float32
BF = mybir.dt.bfloat16
I8 = mybir.dt.int8
AF = mybir.ActivationFunctionType
OP = mybir.AluOpType

H = 4096
QK = 4096
VD = 8192
EPS = 1e-6
NCORES = 8
HPC = 2          # heads per core
RQ = 512         # q/k rows per core
RV = 1024        # v rows / Wo cols per core

_CACHE = {}


def _ensure_ntff_hook():
    """Install the axon NTFF profile hook shim (antenv.axon_hooks is absent
    in this image). Harmless if profiling is never requested."""
    if "antenv.axon_hooks" in sys.modules:
        return
    try:
        import antenv
        mod = types.ModuleType("antenv.axon_hooks")
        mod._hook = None
        mod.set_axon_ntff_profile_hook = lambda h: setattr(mod, "_hook", h)
        mod.get_axon_ntff_profile_hook = lambda: mod._hook
        sys.modules["antenv.axon_hooks"] = mod
        antenv.axon_hooks = mod
        from trn_agent_boot.trn_boot import _ntff_profile_via_ctypes
        mod._hook = _ntff_profile_via_ctypes("/opt/axon/libaxon_pjrt.so")
    except Exception:
        pass


def _build_nc():
    nc = bacc.Bacc(None)

    d = {}
    d["wqk_b"] = nc.dram_tensor("wqk_b", [256, 16384], I8, kind="ExternalInput")
    d["wv_b"] = nc.dram_tensor("wv_b", [256, 16384], I8, kind="ExternalInput")
    d["wo_b"] = nc.dram_tensor("wo_b", [256, 16384], BF, kind="ExternalInput")
    d["wab"] = nc.dram_tensor("wab", [128, 128], F32, kind="ExternalInput")
    d["state_c"] = nc.dram_tensor("state_c", [128, 2048], BF, kind="ExternalInput")
    d["hb"] = nc.dram_tensor("hb", [128, 32], BF, kind="ExternalInput")
    d["h_f32"] = nc.dram_tensor("h_f32", [128, 32], F32, kind="ExternalInput")
    d["qkcache"] = nc.dram_tensor("qkcache", [128, 24], F32, kind="ExternalInput")
    d["qkconvw"] = nc.dram_tensor("qkconvw", [128, 32], F32, kind="ExternalInput")
    d["vcache"] = nc.dram_tensor("vcache", [128, 24], F32, kind="ExternalInput")
    d["vconvw"] = nc.dram_tensor("vconvw", [128, 32], F32, kind="ExternalInput")
    out_d = nc.dram_tensor("out", [1, H], BF, kind="ExternalOutput")

    with tile.TileContext(nc) as tc:
        with (
            tc.tile_pool(name="smalls", bufs=1) as sm,
            tc.tile_pool(name="wpool", bufs=4) as wp,
            tc.tile_pool(name="i8pool", bufs=2) as ip,
            tc.tile_pool(name="psum", bufs=8, space="PSUM") as pm,
        ):
            def emit():
                views = {k: d[k].rearrange("(d p) r -> d p r", p=128)
                         for k in ("wqk_b", "wv_b", "wo_b")}

                def big_tile(key, dd, cuts):
                    # 4 MB SBUF tile streamed in pieces (col offsets in cuts);
                    # finer pieces let consuming matmuls start sooner.
                    t = wp.tile([128, 16384], BF, tag="w", name="wtile")
                    for a, b in zip(cuts, cuts[1:] + [16384]):
                        nc.sync.dma_start(out=t[:, a:b],
                                          in_=views[key][dd][:, a:b])
                    return t

                def big_tile8(key, dd, cuts):
                    # int8 weights: DMA 1 B/elem, then exact on-chip cast to
                    # bf16 (int8 integers are exactly representable). Each
                    # piece's cast is split across vector/scalar/gpsimd
                    # (~237/147/153 G elem/s) so casting keeps pace with DMA.
                    t8 = ip.tile([128, 16384], I8, tag="w8", name="w8tile")
                    tb = wp.tile([128, 16384], BF, tag="w", name="wtile")
                    for a, b in zip(cuts, cuts[1:] + [16384]):
                        nc.sync.dma_start(out=t8[:, a:b],
                                          in_=views[key][dd][:, a:b])
                        c1 = a + ((b - a) * 45 // 100) // 128 * 128
                        c2 = a + ((b - a) * 73 // 100) // 128 * 128
                        nc.vector.tensor_copy(tb[:, a:c1], t8[:, a:c1])
                        nc.scalar.copy(tb[:, c1:c2], t8[:, c1:c2])
                        nc.gpsimd.tensor_copy(tb[:, c2:b], t8[:, c2:b])
                    return tb

                H2 = [0, 8192]
                Q4 = [0, 4096, 8192, 12288]

                # Stream order qk -> v -> o. The whole q/k chain hides
                # under v/o streaming; only the short v tail (fold/conv/SiLU/
                # combine, ~3us) is exposed before the o matmuls. The o DMAs
                # are gated on v1's FIRST half so they issue while v's last
                # 2 MB streams: o bytes arrive right behind v with no bubble,
                # and the o matmuls track the o stream to the end.
                t_qk0 = big_tile8("wqk_b", 0, H2)
                t_qk1 = big_tile8("wqk_b", 1, H2)

                # ---- small input DMAs (SWDGE keeps the HWDGE rings clear) ----
                hb = sm.tile([128, 32], BF, tag="hb")
                hf = sm.tile([128, 32], F32, tag="hf")
                wab = sm.tile([128, 128], F32, tag="wab")
                st = sm.tile([128, 2048], BF, tag="st")
                qkca = sm.tile([128, 24], F32, tag="qkca")
                qkcw = sm.tile([128, 32], F32, tag="qkcw")
                vca = sm.tile([128, 24], F32, tag="vca")
                vcw = sm.tile([128, 32], F32, tag="vcw")
                for t, src in [(hb, "hb"), (wab, "wab"), (hf, "h_f32"),
                               (qkca, "qkcache"), (qkcw, "qkconvw"),
                               (vca, "vcache"), (vcw, "vconvw"),
                               (st, "state_c")]:
                    nc.gpsimd.dma_start(out=t[:], in_=d[src][:])
                ones = sm.tile([1, 128], F32, tag="ones")
                nc.vector.memset(ones[:], 1.0)
                ones_bf = sm.tile([1, 128], BF, tag="ones_bf")
                nc.vector.memset(ones_bf[:], 1.0)
                onesc = sm.tile([128, 1], F32, tag="onesc")
                nc.vector.memset(onesc[:], 1.0)
                epst = sm.tile([1, 1], F32, tag="epst")
                nc.vector.memset(epst[:], EPS)

                # preload the ACT COPY table before Silu/Sqrt so the final
                # psum->sbuf copies don't reload it on the critical path
                cpy_warm = sm.tile([1, 1], F32, tag="cpy_warm")
                nc.scalar.copy(cpy_warm[:], epst[:])

                # ---- conv tap precompute (taps 0-2 need only cached inputs) --
                qacc_p = sm.tile([128, 8], F32, tag="qacc_p")
                qtmp = sm.tile([128, 8], F32, tag="qtmp")
                nc.vector.tensor_mul(qacc_p[:], qkca[:, 0:8], qkcw[:, 0:8])
                for tpi in (1, 2):
                    nc.vector.tensor_mul(qtmp[:], qkca[:, 8 * tpi:8 * tpi + 8],
                                         qkcw[:, 8 * tpi:8 * tpi + 8])
                    nc.vector.tensor_add(qacc_p[:], qacc_p[:], qtmp[:])
                vacc_p = sm.tile([128, 8], F32, tag="vacc_p")
                vtmp = sm.tile([128, 8], F32, tag="vtmp")
                nc.vector.tensor_mul(vacc_p[:], vca[:, 0:8], vcw[:, 0:8])
                for tpi in (1, 2):
                    nc.vector.tensor_mul(vtmp[:], vca[:, 8 * tpi:8 * tpi + 8],
                                         vcw[:, 8 * tpi:8 * tpi + 8])
                    nc.vector.tensor_add(vacc_p[:], vacc_p[:], vtmp[:])

                # ---- big streaming matvecs: one M=1 bf16 matmul per chunk ----
                def mm_piece(t, dd, ps_list, i_lo, i_hi):
                    for i in range(i_lo, i_hi):
                        cc = 16 * dd + i
                        for it, ps in enumerate(ps_list):
                            nc.tensor.matmul(
                                ps[0:1, :], hb[:, cc:cc + 1],
                                t[:, 1024 * i + 512 * it:
                                  1024 * i + 512 * it + 512],
                                start=(cc == 0), stop=(cc == 31))

                # q/k: packed [Wq | Wk]; slot 0 = q, slot 1 = k
                ps_q = pm.tile([1, 512], F32, tag="ps")
                ps_k = pm.tile([1, 512], F32, tag="ps")
                mm_piece(t_qk0, 0, [ps_q, ps_k], 0, 16)
                mm_piece(t_qk1, 1, [ps_q, ps_k], 0, 16)

                # ---- alpha/beta matvec (fp32, tiny); fills the qk->v PE gap
                ps_ab = pm.tile([1, 4], F32, tag="ps")
                for cc in range(32):
                    nc.tensor.matmul(
                        ps_ab[0:1, 0:4], hf[:, cc:cc + 1],
                        wab[:, 4 * cc:4 * cc + 4],
                        start=(cc == 0), stop=(cc == 31))
                # sigmoid(x) = silu(x)/x keeps the ACT table set to
                # {Silu, Sqrt} (2 resident slots -> no reload before v SiLU)
                ab_rec = sm.tile([1, 4], F32, tag="ab_rec")
                nc.vector.reciprocal(ab_rec[:], ps_ab[0:1, :])
                ab_sil = sm.tile([1, 4], F32, tag="ab_sil")
                nc.scalar.activation(ab_sil[:], ps_ab[:], AF.Silu)
                ab = sm.tile([1, 4], F32, tag="ab")
                nc.vector.tensor_mul(ab[:], ab_sil[:], ab_rec[:])

                qrow = sm.tile([1, 512], F32, tag="qrow")
                nc.vector.tensor_copy(qrow[:], ps_q[0:1, :])
                krow = sm.tile([1, 512], F32, tag="krow")
                nc.vector.tensor_copy(krow[:], ps_k[0:1, :])

                # q/k chain tiles, 128-lane column layout (cols 0-3 = k
                # chunks, 4-7 = q chunks); PE pieces injected into the v
                # streaming phase to keep the PE warm through DMA waits.
                t_qk = pm.tile([128, 8], F32, tag="ps")
                qkcol = sm.tile([128, 8], F32, tag="qkcol")
                qacc = sm.tile([128, 8], F32, tag="qacc")
                x1 = sm.tile([128, 8], F32, tag="x1")
                sq = sm.tile([128, 8], F32, tag="sq")
                ps_ss = pm.tile([1, 8], F32, tag="ps")
                ssr = sm.tile([1, 8], F32, tag="ssr")
                ssh = sm.tile([1, 4], F32, tag="ssh")
                srt = sm.tile([1, 4], F32, tag="srt")
                rin = sm.tile([1, 4], F32, tag="rin")
                t_rn = pm.tile([128, 4], F32, tag="ps")
                rbc = sm.tile([128, 4], F32, tag="rbc")
                qkn = sm.tile([128, 8], F32, tag="qkn")
                qkn_bf = sm.tile([128, 8], BF, tag="qkn_bf")
                dm = sm.tile([128, 4], F32, tag="dm")
                ps_dot = pm.tile([1, 4], F32, tag="ps")
                dotr = sm.tile([1, 4], F32, tag="dotr")
                dot = sm.tile([1, 2], F32, tag="dot")
                bd = sm.tile([1, 2], F32, tag="bd")
                abrow = sm.tile([1, 16], F32, tag="abrow")
                t_bc = pm.tile([128, 16], F32, tag="ps")
                abc = sm.tile([128, 16], F32, tag="abc")
                ps_stc = pm.tile([128, 16], F32, tag="ps")
                ksS = sm.tile([128, 8], F32, tag="ksS")
                qsS = sm.tile([128, 8], F32, tag="qsS")

                def chain_pe_0():
                    # raw q/k rows -> columns (K=1 outer products)
                    for c in range(4):
                        nc.tensor.matmul(t_qk[:, c:c + 1],
                                         krow[0:1, 128 * c:128 * c + 128],
                                         ones[0:1, 0:1], start=True, stop=True)
                        nc.tensor.matmul(t_qk[:, 4 + c:5 + c],
                                         qrow[0:1, 128 * c:128 * c + 128],
                                         ones[0:1, 0:1], start=True, stop=True)
                    nc.vector.tensor_copy(qkcol[:], t_qk[:])
                    # conv tap 3 + SiLU in columns
                    nc.vector.tensor_mul(qacc[:], qkcol[:], qkcw[:, 24:32])
                    nc.vector.tensor_add(qacc[:], qacc_p[:], qacc[:])
                    nc.scalar.activation(x1[:], qacc[:], AF.Silu)
                    nc.vector.tensor_mul(sq[:], x1[:], x1[:])

                def chain_pe_1():
                    # per-column sum of squares, then per-head 1/sqrt
                    nc.tensor.matmul(ps_ss[0:1, :], onesc[:, 0:1], sq[:],
                                     start=True, stop=True)
                    nc.vector.tensor_copy(ssr[:], ps_ss[0:1, :])
                    nc.vector.reduce_sum(
                        ssh[0:1, 0:4],
                        ssr[0:1, :].rearrange("a (g t) -> a g t", t=2),
                        axis=mybir.AxisListType.X)
                    nc.scalar.activation(srt[:], ssh[:], AF.Sqrt,
                                         bias=epst[0:1, 0:1])
                    nc.vector.reciprocal(rin[:], srt[:])

                def chain_pe_2():
                    # broadcast 1/norm, normalize columns
                    for j in range(4):
                        nc.tensor.matmul(t_rn[:, j:j + 1], ones[0:1, :],
                                         rin[0:1, j:j + 1], start=True, stop=True)
                    nc.vector.tensor_copy(rbc[:], t_rn[:])
                    for g in range(4):  # k_h0, k_h1, q_h0, q_h1 col pairs
                        nc.vector.tensor_scalar(
                            out=qkn[:, 2 * g:2 * g + 2],
                            in0=x1[:, 2 * g:2 * g + 2],
                            scalar1=rbc[:, g:g + 1], scalar2=None, op0=OP.mult)
                    nc.vector.tensor_copy(qkn_bf[:], qkn[:])
                    # q.k dot per head
                    nc.vector.tensor_mul(dm[:], qkn[:, 4:8], qkn[:, 0:4])
                    nc.tensor.matmul(ps_dot[0:1, :], onesc[:, 0:1], dm[:],
                                     start=True, stop=True)
                    nc.vector.tensor_copy(dotr[:], ps_dot[0:1, :])
                    nc.vector.reduce_sum(
                        dot[0:1, 0:2],
                        dotr[0:1, :].rearrange("a (g t) -> a g t", t=2),
                        axis=mybir.AxisListType.X)
                    nc.vector.tensor_mul(bd[:], ab[0:1, 2:4], dot[0:1, 0:2])
                    # abrow = (a0 a0 a0 a0 a1 a1 a1 a1 | bd0 x4 bd1 x4)
                    for hh in range(HPC):
                        nc.vector.tensor_scalar(
                            out=abrow[0:1, 4 * hh:4 * hh + 4],
                            in0=ones[0:1, 0:4],
                            scalar1=ab[0:1, hh:hh + 1], scalar2=None, op0=OP.mult)
                        nc.vector.tensor_scalar(
                            out=abrow[0:1, 8 + 4 * hh:12 + 4 * hh],
                            in0=ones[0:1, 0:4],
                            scalar1=bd[0:1, hh:hh + 1], scalar2=None, op0=OP.mult)
                    nc.tensor.matmul(t_bc[:, 0:16], ones[0:1, 0:128],
                                     abrow[0:1, 0:16], start=True, stop=True)
                    nc.vector.tensor_copy(abc[:], t_bc[:])

                def chain_pe_3():
                    # state matvecs (bf16 state x bf16 normalized q/k)
                    for hh in range(HPC):
                        for which in range(2):  # 0 -> k, 1 -> q
                            for vc in range(4):
                                col = 8 * which + 4 * hh + vc
                                for d2 in range(2):
                                    blk = 2 * hh + d2
                                    nc.tensor.matmul(
                                        ps_stc[:, col:col + 1],
                                        st[:, 512 * blk + 128 * vc:
                                           512 * blk + 128 * vc + 128],
                                        qkn_bf[:, 4 * which + 2 * hh + d2:
                                               4 * which + 2 * hh + d2 + 1],
                                        start=(d2 == 0), stop=(d2 == 1))
                    # alpha-scaled retrieved/out state (combine precompute)
                    nc.vector.tensor_mul(ksS[:], ps_stc[:, 0:8], abc[:, 0:8])
                    nc.vector.tensor_mul(qsS[:], ps_stc[:, 8:16], abc[:, 0:8])

                # ---- v matvec; chain PE work injected between its pieces ----
                ps_v0 = pm.tile([1, 512], F32, tag="ps")
                ps_v1 = pm.tile([1, 512], F32, tag="ps")
                ps_warm = pm.tile([1, 512], F32, tag="ps", name="ps_warm")
                t_v0 = big_tile8("wv_b", 0, H2)
                mm_piece(t_v0, 0, [ps_v0, ps_v1], 0, 8)
                chain_pe_0()
                mm_piece(t_v0, 0, [ps_v0, ps_v1], 8, 16)
                chain_pe_1()
                # v1 streams in 1 MB pieces: at the v/o boundary it holds
                # most of the in-flight DMA slots, so round-robin service
                # gives v's tail most of the bandwidth without any gate.
                t_v1 = big_tile8("wv_b", 1, Q4)

                def warm_mms(n):
                    for _ in range(n):
                        nc.tensor.matmul(ps_warm[0:1, :], hb[:, 0:1],
                                         t_v0[:, 0:512], start=True, stop=True)

                mm_piece(t_v1, 1, [ps_v0, ps_v1], 0, 4)
                chain_pe_2()
                mm_piece(t_v1, 1, [ps_v0, ps_v1], 4, 8)
                chain_pe_3()
                mm_piece(t_v1, 1, [ps_v0, ps_v1], 8, 12)
                warm_mms(3)
                mm_piece(t_v1, 1, [ps_v0, ps_v1], 12, 16)

                # ---- v tail: rows -> bf16 -> columns -> conv -> SiLU ----
                vsb = sm.tile([1, 1024], BF, tag="vsb")
                nc.vector.tensor_copy(vsb[0:1, 0:512], ps_v0[0:1, :])
                nc.vector.tensor_copy(vsb[0:1, 512:1024], ps_v1[0:1, :])
                t_v = pm.tile([128, 8], F32, tag="ps")
                for j in range(8):
                    nc.tensor.matmul(t_v[:, j:j + 1],
                                     vsb[0:1, 128 * j:128 * j + 128],
                                     ones_bf[0:1, 0:1], start=True, stop=True)
                warm_mms(4)
                vcol = sm.tile([128, 8], F32, tag="vcol")
                nc.vector.tensor_copy(vcol[:], t_v[:])
                vacc = sm.tile([128, 8], F32, tag="vacc")
                nc.vector.tensor_mul(vacc[:], vcol[:], vcw[:, 24:32])
                nc.vector.tensor_add(vacc[:], vacc_p[:], vacc[:])
                v1c = sm.tile([128, 8], F32, tag="v1c")
                nc.scalar.activation(v1c[:], vacc[:], AF.Silu)

                # ---- combine: ov = a*qs + (b*dot)*(v1 - a*ks) ----
                ovc = sm.tile([128, 8], F32, tag="ovc")
                err8 = sm.tile([128, 8], F32, tag="err8")
                nc.vector.tensor_sub(err8[:], v1c[:], ksS[:])
                nc.vector.tensor_mul(err8[:], err8[:], abc[:, 8:16])
                nc.vector.tensor_add(ovc[:], qsS[:], err8[:])
                ov_hi = sm.tile([128, 8], BF, tag="ov_hi")
                nc.vector.tensor_copy(ov_hi[:], ovc[:])

                # ---- output projection ----
                ps_o = [pm.tile([1, 512], F32, tag="ps", name=f"ps_o{i}")
                        for i in range(8)]
                out_sb = sm.tile([1, H], BF, tag="out_sb")
                for dd in range(2):
                    t = big_tile("wo_b", dd, Q4 if dd == 0 else [0, 8192, 12288, 14336])
                    for i in range(4):
                        j = 4 * dd + i
                        for it in range(8):
                            sl = slice(4096 * i + 512 * it,
                                       4096 * i + 512 * it + 512)
                            nc.tensor.matmul(ps_o[it][0:1, :], ov_hi[:, j:j + 1],
                                             t[:, sl], start=(j == 0),
                                             stop=(j == 7))
                for half in range(2):
                    for it in range(4 * half, 4 * half + 4):
                        dst = out_sb[0:1, 512 * it:512 * it + 512]
                        if it % 2 == 0:
                            nc.vector.tensor_copy(dst, ps_o[it][0:1, :])
                        else:
                            nc.scalar.copy(dst, ps_o[it][0:1, :])
                    sl = slice(2048 * half, 2048 * half + 2048)
                    nc.sync.dma_start(out=out_d[0:1, sl], in_=out_sb[0:1, sl])

            emit()

    nc.finalize()
    return nc


def _tile_major(mT):
    """mT [R, C] bf16 -> packed so each 128-row device tile is one
    contiguous 32 KB span per partition. ni = 16384//C."""
    R, C = mT.shape
    ni = 16384 // C
    dd = R // (128 * ni)
    return np.ascontiguousarray(
        mT.reshape(dd, ni, 128, C).transpose(0, 2, 1, 3).reshape(dd * 128, ni * C))


def _prep_in_maps(inputs):
    f32 = np.float32
    hid = np.asarray(inputs["hidden_states"], f32)[0, :, 0, 0]     # [4096]
    Wq = np.asarray(inputs["Wq"], f32)
    Wk = np.asarray(inputs["Wk"], f32)
    Wv = np.asarray(inputs["Wv"], f32)
    Wo = np.asarray(inputs["Wo"], f32)
    Wa = np.asarray(inputs["Wa"], f32)
    Wb = np.asarray(inputs["Wb"], f32)
    qcw = np.asarray(inputs["q_conv_w"], f32)[0]                   # [QK, 4]
    kcw = np.asarray(inputs["k_conv_w"], f32)[0]
    vcw = np.asarray(inputs["v_conv_w"], f32)[0]                   # [VD, 4]
    qca = np.asarray(inputs["q_cache"], f32)[0]                    # [QK, 3]
    kca = np.asarray(inputs["k_cache"], f32)[0]
    vca = np.asarray(inputs["v_cache"], f32)[0]                    # [VD, 3]
    state = np.asarray(inputs["state"], f32)[0]                    # [16,256,512]

    cols = lambda v: np.ascontiguousarray(v.reshape(32, 128).T)
    hb_c = cols(hid.astype(BF16))
    h_f_c = cols(hid)

    in_maps = []
    for c in range(NCORES):
        rq = slice(c * RQ, (c + 1) * RQ)
        rv = slice(c * RV, (c + 1) * RV)
        # packed [Wq ; Wk] rows -> int8 with per-row scales; the scales
        # ride in the tap-3 conv weights (q1 = sum caches*w + (s*q_raw)*w3)
        wqk = np.concatenate([Wq[rq], Wk[rq]], axis=0)             # [1024, 4096]
        s_qk = np.abs(wqk).max(axis=1) / 127.0                     # [1024]
        wqk8 = np.round(wqk / s_qk[:, None]).astype(np.int8)
        wqk_b = _tile_major(np.ascontiguousarray(wqk8.T))
        wv = Wv[rv]                                                # [1024, 4096]
        s_v = np.abs(wv).max(axis=1) / 127.0
        wv8 = np.round(wv / s_v[:, None]).astype(np.int8)
        wv_b = _tile_major(np.ascontiguousarray(wv8.T))
        wo_b = _tile_major(np.ascontiguousarray(Wo[:, rv]).astype(BF16).T.copy())

        wab = np.concatenate([Wa[2 * c:2 * c + 2], Wb[2 * c:2 * c + 2]], 0)
        wab_sb = np.ascontiguousarray(
            wab.reshape(4, 32, 128).transpose(2, 1, 0).reshape(128, 128))
        st_sb = np.ascontiguousarray(
            state[2 * c:2 * c + 2].reshape(2, 2, 128, 512)
            .transpose(2, 0, 1, 3).reshape(128, 2048)).astype(BF16)

        # q/k conv in column layout [128, 8*taps]: per tap, cols 0-3 = k
        # chunks (k idx 128c+p), cols 4-7 = q chunks
        qk_ca = np.concatenate(
            [np.concatenate([kca[rq, t].reshape(4, 128).T,
                             qca[rq, t].reshape(4, 128).T], 1)
             for t in range(3)], 1)
        # tap 3 multiplies the raw int8 matvec result, so fold the int8
        # per-row scales into it (cols 0-3 = k chunks, 4-7 = q chunks)
        s_q_c = s_qk[0:512].reshape(4, 128).T
        s_k_c = s_qk[512:1024].reshape(4, 128).T
        qk_cw = np.concatenate(
            [np.concatenate([kcw[rq, t].reshape(4, 128).T * (s_k_c if t == 3 else 1.0),
                             qcw[rq, t].reshape(4, 128).T * (s_q_c if t == 3 else 1.0)], 1)
             for t in range(4)], 1)
        # v conv in column layout [128, 8*taps]: vcol[p, 8t+cc] = v[128cc+p, t]
        v_ca = np.ascontiguousarray(
            vca[rv].reshape(8, 128, 3).transpose(1, 2, 0).reshape(128, 24))
        v_cw = vcw[rv].reshape(8, 128, 4).transpose(1, 2, 0).reshape(128, 32).copy()
        v_cw[:, 24:32] *= s_v.reshape(8, 128).T

        in_maps.append({
            "wqk_b": wqk_b, "wv_b": wv_b, "wo_b": wo_b,
            "wab": wab_sb, "state_c": st_sb,
            "hb": hb_c, "h_f32": h_f_c,
            "qkcache": np.ascontiguousarray(qk_ca),
            "qkconvw": np.ascontiguousarray(qk_cw),
            "vcache": v_ca, "vconvw": v_cw,
        })
    return in_maps


def _run(inputs, trace=False, tmpdir=None):
    _ensure_ntff_hook()
    if "nc" not in _CACHE:
        _CACHE["nc"] = _build_nc()
    nc = _CACHE["nc"]
    in_maps = _prep_in_maps(inputs)
    res = run_bass_kernel_spmd(nc, in_maps, list(range(NCORES)),
                               trace=trace, tmpdir=tmpdir)
    acc = np.zeros(H, np.float64)
    for c in range(NCORES):
        acc += res.results[c]["out"][0].astype(np.float64)
    out = acc.astype(np.float32).reshape(1, H, 1, 1)
    return out, res


def kernel(**inputs):
    out, _ = _run(inputs, trace=False)
    return out


def kernel_traced(tmpdir=None, **inputs):
    return _run(inputs, trace=True, tmpdir=tmpdir)


# revision 39
# speedup vs baseline: 1.5935x; 1.5935x over previous
"""DeltaNet decode step on 8 Trainium2 NeuronCores (tensor-parallel over heads).

Contract: kernel(**inputs) takes the FULL unsharded inputs (numpy arrays,
same keys as the reference setup_inputs()) and returns the FULL output
[1, 4096, 1, 1] float32.

Sharding (8 cores, 16 heads -> 2 heads/core):
  - Wq/Wk rows, q/k conv weights+caches: 512 rows per core
  - Wv rows, v conv weights+caches, Wo columns: 1024 per core
  - state: 2 heads per core
  - output: each core computes a partial [4096] projection; host all-reduces.

Device kernel: memory-bound mat-vec streaming. Weights are host-converted
to single bf16 and packed tile-major (each [128, 16384] SBUF tile is one
contiguous 32 KB row per partition in DRAM). wpool bufs=2 serializes the
weight DMAs into phase order (qk, v, o) so later phases don't steal HBM
bandwidth from earlier ones; the last phases stream in 1 MB quarters so
the final matmuls lag the last byte by only ~2 us.

The post-matvec chain (conv, l2norm, state update, combine) runs in
128-lane column layout. Everything that doesn't depend on v (alpha/beta
broadcast, state matvecs, alpha*ks / alpha*qs) is precomputed during the
v/o streaming phases, leaving a ~4 us serial tail: v fold -> conv tap 3
-> SiLU -> 3 wide DVE ops -> bf16 cast.
"""

import sys
import types

sys.path.insert(0, "/opt/trn_rl_repo")

import numpy as np
import ml_dtypes

import concourse.bass as bass
import concourse.mybir as mybir
import concourse.tile as tile
from concourse import bacc
from concourse.bass_utils import run_bass_kernel_spmd

BF16 = ml_dtypes.bfloat16
F32 = mybir.dt.float32
BF = mybir.dt.bfloat16
I8 = mybir.dt.int8
AF = mybir.ActivationFunctionType
OP = mybir.AluOpType

H = 4096
QK = 4096
VD = 8192
EPS = 1e-6
NCORES = 8
HPC = 2          # heads per core
RQ = 512         # q/k rows per core
RV = 1024        # v rows / Wo cols per core

_CACHE = {}


def _ensure_ntff_hook():
    """Install the axon NTFF profile hook shim (antenv.axon_hooks is absent
    in this image). Harmless if profiling is never requested."""
    if "antenv.axon_hooks" in sys.modules:
        return
    try:
        import antenv
        mod = types.ModuleType("antenv.axon_hooks")
        mod._hook = None
        mod.set_axon_ntff_profile_hook = lambda h: setattr(mod, "_hook", h)
        mod.get_axon_ntff_profile_hook = lambda: mod._hook
        sys.modules["antenv.axon_hooks"] = mod
        antenv.axon_hooks = mod
        from trn_agent_boot.trn_boot import _ntff_profile_via_ctypes
        mod._hook = _ntff_profile_via_ctypes("/opt/axon/libaxon_pjrt.so")
    except Exception:
        pass


def _build_nc():
    nc = bacc.Bacc(None)

    d = {}
    d["wqk_b"] = nc.dram_tensor("wqk_b", [256, 16384], I8, kind="ExternalInput")
    d["wv_b"] = nc.dram_tensor("wv_b", [256, 16384], I8, kind="ExternalInput")
    d["wo_b"] = nc.dram_tensor("wo_b", [256, 16384], BF, kind="ExternalInput")
    d["wab"] = nc.dram_tensor("wab", [128, 128], F32, kind="ExternalInput")
    d["state_c"] = nc.dram_tensor("state_c", [128, 2048], BF, kind="ExternalInput")
    d["hb"] = nc.dram_tensor("hb", [128, 32], BF, kind="ExternalInput")
    d["h_f32"] = nc.dram_tensor("h_f32", [128, 32], F32, kind="ExternalInput")
    d["qkcache"] = nc.dram_tensor("qkcache", [128, 24], F32, kind="ExternalInput")
    d["qkconvw"] = nc.dram_tensor("qkconvw", [128, 32], F32, kind="ExternalInput")
    d["vcache"] = nc.dram_tensor("vcache", [128, 24], F32, kind="ExternalInput")
    d["vconvw"] = nc.dram_tensor("vconvw", [128, 32], F32, kind="ExternalInput")
    out_d = nc.dram_tensor("out", [1, H], BF, kind="ExternalOutput")

    with tile.TileContext(nc) as tc:
        with (
            tc.tile_pool(name="smalls", bufs=1) as sm,
            tc.tile_pool(name="wpool", bufs=4) as wp,
            tc.tile_pool(name="i8pool", bufs=2) as ip,
            tc.tile_pool(name="psum", bufs=8, space="PSUM") as pm,
        ):
            def emit():
                views = {k: d[k].rearrange("(d p) r -> d p r", p=128)
                         for k in ("wqk_b", "wv_b", "wo_b")}

                def big_tile(key, dd, cuts):
                    # 4 MB SBUF tile streamed in pieces (col offsets in cuts);
                    # finer pieces let consuming matmuls start sooner.
                    t = wp.tile([128, 16384], BF, tag="w", name="wtile")
                    for a, b in zip(cuts, cuts[1:] + [16384]):
                        nc.sync.dma_start(out=t[:, a:b],
                                          in_=views[key][dd][:, a:b])
                    return t

                def big_tile8(key, dd, cuts):
                    # int8 weights: DMA 1 B/elem, then exact on-chip cast to
                    # bf16 (int8 integers are exactly representable). Each
                    # piece's cast is split across vector/scalar/gpsimd
                    # (~237/147/153 G elem/s) so casting keeps pace with DMA.
                    t8 = ip.tile([128, 16384], I8, tag="w8", name="w8tile")
                    tb = wp.tile([128, 16384], BF, tag="w", name="wtile")
                    for a, b in zip(cuts, cuts[1:] + [16384]):
                        nc.sync.dma_start(out=t8[:, a:b],
                                          in_=views[key][dd][:, a:b])
                        c1 = a + ((b - a) * 62 // 100) // 128 * 128
                        nc.vector.tensor_copy(tb[:, a:c1], t8[:, a:c1])
                        nc.scalar.copy(tb[:, c1:b], t8[:, c1:b])
                    return tb

                H2 = [0, 8192]
                Q4 = [0, 4096, 8192, 12288]

                # Stream order qk -> v -> o. The whole q/k chain hides
                # under v/o streaming; only the short v tail (fold/conv/SiLU/
                # combine, ~3us) is exposed before the o matmuls. The o DMAs
                # are gated on v1's FIRST half so they issue while v's last
                # 2 MB streams: o bytes arrive right behind v with no bubble,
                # and the o matmuls track the o stream to the end.
                t_qk0 = big_tile8("wqk_b", 0, H2)
                t_qk1 = big_tile8("wqk_b", 1, H2)

                # ---- small input DMAs (SWDGE keeps the HWDGE rings clear) ----
                hb = sm.tile([128, 32], BF, tag="hb")
                hf = sm.tile([128, 32], F32, tag="hf")
                wab = sm.tile([128, 128], F32, tag="wab")
                st = sm.tile([128, 2048], BF, tag="st")
                qkca = sm.tile([128, 24], F32, tag="qkca")
                qkcw = sm.tile([128, 32], F32, tag="qkcw")
                vca = sm.tile([128, 24], F32, tag="vca")
                vcw = sm.tile([128, 32], F32, tag="vcw")
                for t, src in [(hb, "hb"), (wab, "wab"), (hf, "h_f32"),
                               (qkca, "qkcache"), (qkcw, "qkconvw"),
                               (vca, "vcache"), (vcw, "vconvw"),
                               (st, "state_c")]:
                    nc.gpsimd.dma_start(out=t[:], in_=d[src][:])
                ones = sm.tile([1, 128], F32, tag="ones")
                nc.vector.memset(ones[:], 1.0)
                ones_bf = sm.tile([1, 128], BF, tag="ones_bf")
                nc.vector.memset(ones_bf[:], 1.0)
                onesc = sm.tile([128, 1], F32, tag="onesc")
                nc.vector.memset(onesc[:], 1.0)
                epst = sm.tile([1, 1], F32, tag="epst")
                nc.vector.memset(epst[:], EPS)

                # preload the ACT COPY table before Silu/Sqrt so the final
                # psum->sbuf copies don't reload it on the critical path
                cpy_warm = sm.tile([1, 1], F32, tag="cpy_warm")
                nc.scalar.copy(cpy_warm[:], epst[:])

                # ---- conv tap precompute (taps 0-2 need only cached inputs) --
                qacc_p = sm.tile([128, 8], F32, tag="qacc_p")
                qtmp = sm.tile([128, 8], F32, tag="qtmp")
                nc.vector.tensor_mul(qacc_p[:], qkca[:, 0:8], qkcw[:, 0:8])
                for tpi in (1, 2):
                    nc.vector.tensor_mul(qtmp[:], qkca[:, 8 * tpi:8 * tpi + 8],
                                         qkcw[:, 8 * tpi:8 * tpi + 8])
                    nc.vector.tensor_add(qacc_p[:], qacc_p[:], qtmp[:])
                vacc_p = sm.tile([128, 8], F32, tag="vacc_p")
                vtmp = sm.tile([128, 8], F32, tag="vtmp")
                nc.vector.tensor_mul(vacc_p[:], vca[:, 0:8], vcw[:, 0:8])
                for tpi in (1, 2):
                    nc.vector.tensor_mul(vtmp[:], vca[:, 8 * tpi:8 * tpi + 8],
                                         vcw[:, 8 * tpi:8 * tpi + 8])
                    nc.vector.tensor_add(vacc_p[:], vacc_p[:], vtmp[:])

                # ---- big streaming matvecs: one M=1 bf16 matmul per chunk ----
                def mm_piece(t, dd, ps_list, i_lo, i_hi):
                    for i in range(i_lo, i_hi):
                        cc = 16 * dd + i
                        for it, ps in enumerate(ps_list):
                            nc.tensor.matmul(
                                ps[0:1, :], hb[:, cc:cc + 1],
                                t[:, 1024 * i + 512 * it:
                                  1024 * i + 512 * it + 512],
                                start=(cc == 0), stop=(cc == 31))

                # q/k: packed [Wq | Wk]; slot 0 = q, slot 1 = k
                ps_q = pm.tile([1, 512], F32, tag="ps")
                ps_k = pm.tile([1, 512], F32, tag="ps")
                mm_piece(t_qk0, 0, [ps_q, ps_k], 0, 16)
                mm_piece(t_qk1, 1, [ps_q, ps_k], 0, 16)

                # ---- alpha/beta matvec (fp32, tiny); fills the qk->v PE gap
                ps_ab = pm.tile([1, 4], F32, tag="ps")
                for cc in range(32):
                    nc.tensor.matmul(
                        ps_ab[0:1, 0:4], hf[:, cc:cc + 1],
                        wab[:, 4 * cc:4 * cc + 4],
                        start=(cc == 0), stop=(cc == 31))
                # sigmoid(x) = silu(x)/x keeps the ACT table set to
                # {Silu, Sqrt} (2 resident slots -> no reload before v SiLU)
                ab_rec = sm.tile([1, 4], F32, tag="ab_rec")
                nc.vector.reciprocal(ab_rec[:], ps_ab[0:1, :])
                ab_sil = sm.tile([1, 4], F32, tag="ab_sil")
                nc.scalar.activation(ab_sil[:], ps_ab[:], AF.Silu)
                ab = sm.tile([1, 4], F32, tag="ab")
                nc.vector.tensor_mul(ab[:], ab_sil[:], ab_rec[:])

                qrow = sm.tile([1, 512], F32, tag="qrow")
                nc.vector.tensor_copy(qrow[:], ps_q[0:1, :])
                krow = sm.tile([1, 512], F32, tag="krow")
                nc.vector.tensor_copy(krow[:], ps_k[0:1, :])

                # q/k chain tiles, 128-lane column layout (cols 0-3 = k
                # chunks, 4-7 = q chunks); PE pieces injected into the v
                # streaming phase to keep the PE warm through DMA waits.
                t_qk = pm.tile([128, 8], F32, tag="ps")
                qkcol = sm.tile([128, 8], F32, tag="qkcol")
                qacc = sm.tile([128, 8], F32, tag="qacc")
                x1 = sm.tile([128, 8], F32, tag="x1")
                sq = sm.tile([128, 8], F32, tag="sq")
                ps_ss = pm.tile([1, 8], F32, tag="ps")
                ssr = sm.tile([1, 8], F32, tag="ssr")
                ssh = sm.tile([1, 4], F32, tag="ssh")
                srt = sm.tile([1, 4], F32, tag="srt")
                rin = sm.tile([1, 4], F32, tag="rin")
                t_rn = pm.tile([128, 4], F32, tag="ps")
                rbc = sm.tile([128, 4], F32, tag="rbc")
                qkn = sm.tile([128, 8], F32, tag="qkn")
                qkn_bf = sm.tile([128, 8], BF, tag="qkn_bf")
                dm = sm.tile([128, 4], F32, tag="dm")
                ps_dot = pm.tile([1, 4], F32, tag="ps")
                dotr = sm.tile([1, 4], F32, tag="dotr")
                dot = sm.tile([1, 2], F32, tag="dot")
                bd = sm.tile([1, 2], F32, tag="bd")
                abrow = sm.tile([1, 16], F32, tag="abrow")
                t_bc = pm.tile([128, 16], F32, tag="ps")
                abc = sm.tile([128, 16], F32, tag="abc")
                ps_stc = pm.tile([128, 16], F32, tag="ps")
                ksS = sm.tile([128, 8], F32, tag="ksS")
                qsS = sm.tile([128, 8], F32, tag="qsS")

                def chain_pe_0():
                    # raw q/k rows -> columns (K=1 outer products)
                    for c in range(4):
                        nc.tensor.matmul(t_qk[:, c:c + 1],
                                         krow[0:1, 128 * c:128 * c + 128],
                                         ones[0:1, 0:1], start=True, stop=True)
                        nc.tensor.matmul(t_qk[:, 4 + c:5 + c],
                                         qrow[0:1, 128 * c:128 * c + 128],
                                         ones[0:1, 0:1], start=True, stop=True)
                    nc.vector.tensor_copy(qkcol[:], t_qk[:])
                    # conv tap 3 + SiLU in columns
                    nc.vector.tensor_mul(qacc[:], qkcol[:], qkcw[:, 24:32])
                    nc.vector.tensor_add(qacc[:], qacc_p[:], qacc[:])
                    nc.scalar.activation(x1[:], qacc[:], AF.Silu)
                    nc.vector.tensor_mul(sq[:], x1[:], x1[:])

                def chain_pe_1():
                    # per-column sum of squares, then per-head 1/sqrt
                    nc.tensor.matmul(ps_ss[0:1, :], onesc[:, 0:1], sq[:],
                                     start=True, stop=True)
                    nc.vector.tensor_copy(ssr[:], ps_ss[0:1, :])
                    nc.vector.reduce_sum(
                        ssh[0:1, 0:4],
                        ssr[0:1, :].rearrange("a (g t) -> a g t", t=2),
                        axis=mybir.AxisListType.X)
                    nc.scalar.activation(srt[:], ssh[:], AF.Sqrt,
                                         bias=epst[0:1, 0:1])
                    nc.vector.reciprocal(rin[:], srt[:])

                def chain_pe_2():
                    # broadcast 1/norm, normalize columns
                    for j in range(4):
                        nc.tensor.matmul(t_rn[:, j:j + 1], ones[0:1, :],
                                         rin[0:1, j:j + 1], start=True, stop=True)
                    nc.vector.tensor_copy(rbc[:], t_rn[:])
                    for g in range(4):  # k_h0, k_h1, q_h0, q_h1 col pairs
                        nc.vector.tensor_scalar(
                            out=qkn[:, 2 * g:2 * g + 2],
                            in0=x1[:, 2 * g:2 * g + 2],
                            scalar1=rbc[:, g:g + 1], scalar2=None, op0=OP.mult)
                    nc.vector.tensor_copy(qkn_bf[:], qkn[:])
                    # q.k dot per head
                    nc.vector.tensor_mul(dm[:], qkn[:, 4:8], qkn[:, 0:4])
                    nc.tensor.matmul(ps_dot[0:1, :], onesc[:, 0:1], dm[:],
                                     start=True, stop=True)
                    nc.vector.tensor_copy(dotr[:], ps_dot[0:1, :])
                    nc.vector.reduce_sum(
                        dot[0:1, 0:2],
                        dotr[0:1, :].rearrange("a (g t) -> a g t", t=2),
                        axis=mybir.AxisListType.X)
                    nc.vector.tensor_mul(bd[:], ab[0:1, 2:4], dot[0:1, 0:2])
                    # abrow = (a0 a0 a0 a0 a1 a1 a1 a1 | bd0 x4 bd1 x4)
                    for hh in range(HPC):
                        nc.vector.tensor_scalar(
                            out=abrow[0:1, 4 * hh:4 * hh + 4],
                            in0=ones[0:1, 0:4],
                            scalar1=ab[0:1, hh:hh + 1], scalar2=None, op0=OP.mult)
                        nc.vector.tensor_scalar(
                            out=abrow[0:1, 8 + 4 * hh:12 + 4 * hh],
                            in0=ones[0:1, 0:4],
                            scalar1=bd[0:1, hh:hh + 1], scalar2=None, op0=OP.mult)
                    nc.tensor.matmul(t_bc[:, 0:16], ones[0:1, 0:128],
                                     abrow[0:1, 0:16], start=True, stop=True)
                    nc.vector.tensor_copy(abc[:], t_bc[:])

                def chain_pe_3():
                    # state matvecs (bf16 state x bf16 normalized q/k)
                    for hh in range(HPC):
                        for which in range(2):  # 0 -> k, 1 -> q
                            for vc in range(4):
                                col = 8 * which + 4 * hh + vc
                                for d2 in range(2):
                                    blk = 2 * hh + d2
                                    nc.tensor.matmul(
                                        ps_stc[:, col:col + 1],
                                        st[:, 512 * blk + 128 * vc:
                                           512 * blk + 128 * vc + 128],
                                        qkn_bf[:, 4 * which + 2 * hh + d2:
                                               4 * which + 2 * hh + d2 + 1],
                                        start=(d2 == 0), stop=(d2 == 1))
                    # alpha-scaled retrieved/out state (combine precompute)
                    nc.vector.tensor_mul(ksS[:], ps_stc[:, 0:8], abc[:, 0:8])
                    nc.vector.tensor_mul(qsS[:], ps_stc[:, 8:16], abc[:, 0:8])

                # ---- v matvec; chain PE work injected between its pieces ----
                ps_v0 = pm.tile([1, 512], F32, tag="ps")
                ps_v1 = pm.tile([1, 512], F32, tag="ps")
                ps_warm = pm.tile([1, 512], F32, tag="ps", name="ps_warm")
                t_v0 = big_tile8("wv_b", 0, H2)
                mm_piece(t_v0, 0, [ps_v0, ps_v1], 0, 8)
                chain_pe_0()
                mm_piece(t_v0, 0, [ps_v0, ps_v1], 8, 16)
                chain_pe_1()
                # v1 streams in 1 MB pieces: at the v/o boundary it holds
                # most of the in-flight DMA slots, so round-robin service
                # gives v's tail most of the bandwidth without any gate.
                t_v1 = big_tile8("wv_b", 1, Q4)

                def warm_mms(n):
                    for _ in range(n):
                        nc.tensor.matmul(ps_warm[0:1, :], hb[:, 0:1],
                                         t_v0[:, 0:512], start=True, stop=True)

                mm_piece(t_v1, 1, [ps_v0, ps_v1], 0, 4)
                chain_pe_2()
                mm_piece(t_v1, 1, [ps_v0, ps_v1], 4, 8)
                chain_pe_3()
                mm_piece(t_v1, 1, [ps_v0, ps_v1], 8, 12)
                warm_mms(3)
                mm_piece(t_v1, 1, [ps_v0, ps_v1], 12, 16)

                # ---- v tail: rows -> bf16 -> columns -> conv -> SiLU ----
                vsb = sm.tile([1, 1024], BF, tag="vsb")
                nc.vector.tensor_copy(vsb[0:1, 0:512], ps_v0[0:1, :])
                nc.vector.tensor_copy(vsb[0:1, 512:1024], ps_v1[0:1, :])
                t_v = pm.tile([128, 8], F32, tag="ps")
                for j in range(8):
                    nc.tensor.matmul(t_v[:, j:j + 1],
                                     vsb[0:1, 128 * j:128 * j + 128],
                                     ones_bf[0:1, 0:1], start=True, stop=True)
                warm_mms(4)
                vcol = sm.tile([128, 8], F32, tag="vcol")
                nc.vector.tensor_copy(vcol[:], t_v[:])
                vacc = sm.tile([128, 8], F32, tag="vacc")
                nc.vector.tensor_mul(vacc[:], vcol[:], vcw[:, 24:32])
                nc.vector.tensor_add(vacc[:], vacc_p[:], vacc[:])
                v1c = sm.tile([128, 8], F32, tag="v1c")
                nc.scalar.activation(v1c[:], vacc[:], AF.Silu)

                # ---- combine: ov = a*qs + (b*dot)*(v1 - a*ks) ----
                ovc = sm.tile([128, 8], F32, tag="ovc")
                err8 = sm.tile([128, 8], F32, tag="err8")
                nc.vector.tensor_sub(err8[:], v1c[:], ksS[:])
                nc.vector.tensor_mul(err8[:], err8[:], abc[:, 8:16])
                nc.vector.tensor_add(ovc[:], qsS[:], err8[:])
                ov_hi = sm.tile([128, 8], BF, tag="ov_hi")
                nc.vector.tensor_copy(ov_hi[:], ovc[:])

                # ---- output projection ----
                ps_o = [pm.tile([1, 512], F32, tag="ps", name=f"ps_o{i}")
                        for i in range(8)]
                out_sb = sm.tile([1, H], BF, tag="out_sb")
                for dd in range(2):
                    t = big_tile("wo_b", dd, Q4 if dd == 0 else [0, 8192, 12288, 14336])
                    for i in range(4):
                        j = 4 * dd + i
                        for it in range(8):
                            sl = slice(4096 * i + 512 * it,
                                       4096 * i + 512 * it + 512)
                            nc.tensor.matmul(ps_o[it][0:1, :], ov_hi[:, j:j + 1],
                                             t[:, sl], start=(j == 0),
                                             stop=(j == 7))
                for half in range(2):
                    for it in range(4 * half, 4 * half + 4):
                        dst = out_sb[0:1, 512 * it:512 * it + 512]
                        if it % 2 == 0:
                            nc.vector.tensor_copy(dst, ps_o[it][0:1, :])
                        else:
                            nc.scalar.copy(dst, ps_o[it][0:1, :])
                    sl = slice(2048 * half, 2048 * half + 2048)
                    nc.sync.dma_start(out=out_d[0:1, sl], in_=out_sb[0:1, sl])

            emit()

    nc.finalize()
    return nc


def _tile_major(mT):
    """mT [R, C] bf16 -> packed so each 128-row device tile is one
    contiguous 32 KB span per partition. ni = 16384//C."""
    R, C = mT.shape
    ni = 16384 // C
    dd = R // (128 * ni)
    return np.ascontiguousarray(
        mT.reshape(dd, ni, 128, C).transpose(0, 2, 1, 3).reshape(dd * 128, ni * C))


def _prep_in_maps(inputs):
    f32 = np.float32
    hid = np.asarray(inputs["hidden_states"], f32)[0, :, 0, 0]     # [4096]
    Wq = np.asarray(inputs["Wq"], f32)
    Wk = np.asarray(inputs["Wk"], f32)
    Wv = np.asarray(inputs["Wv"], f32)
    Wo = np.asarray(inputs["Wo"], f32)
    Wa = np.asarray(inputs["Wa"], f32)
    Wb = np.asarray(inputs["Wb"], f32)
    qcw = np.asarray(inputs["q_conv_w"], f32)[0]                   # [QK, 4]
    kcw = np.asarray(inputs["k_conv_w"], f32)[0]
    vcw = np.asarray(inputs["v_conv_w"], f32)[0]                   # [VD, 4]
    qca = np.asarray(inputs["q_cache"], f32)[0]                    # [QK, 3]
    kca = np.asarray(inputs["k_cache"], f32)[0]
    vca = np.asarray(inputs["v_cache"], f32)[0]                    # [VD, 3]
    state = np.asarray(inputs["state"], f32)[0]                    # [16,256,512]

    cols = lambda v: np.ascontiguousarray(v.reshape(32, 128).T)
    hb_c = cols(hid.astype(BF16))
    h_f_c = cols(hid)

    in_maps = []
    for c in range(NCORES):
        rq = slice(c * RQ, (c + 1) * RQ)
        rv = slice(c * RV, (c + 1) * RV)
        # packed [Wq ; Wk] rows -> int8 with per-row scales; the scales
        # ride in the tap-3 conv weights (q1 = sum caches*w + (s*q_raw)*w3)
        wqk = np.concatenate([Wq[rq], Wk[rq]], axis=0)             # [1024, 4096]
        s_qk = np.abs(wqk).max(axis=1) / 127.0                     # [1024]
        wqk8 = np.round(wqk / s_qk[:, None]).astype(np.int8)
        wqk_b = _tile_major(np.ascontiguousarray(wqk8.T))
        wv = Wv[rv]                                                # [1024, 4096]
        s_v = np.abs(wv).max(axis=1) / 127.0
        wv8 = np.round(wv / s_v[:, None]).astype(np.int8)
        wv_b = _tile_major(np.ascontiguousarray(wv8.T))
        wo_b = _tile_major(np.ascontiguousarray(Wo[:, rv]).astype(BF16).T.copy())

        wab = np.concatenate([Wa[2 * c:2 * c + 2], Wb[2 * c:2 * c + 2]], 0)
        wab_sb = np.ascontiguousarray(
            wab.reshape(4, 32, 128).transpose(2, 1, 0).reshape(128, 128))
        st_sb = np.ascontiguousarray(
            state[2 * c:2 * c + 2].reshape(2, 2, 128, 512)
            .transpose(2, 0, 1, 3).reshape(128, 2048)).astype(BF16)

        # q/k conv in column layout [128, 8*taps]: per tap, cols 0-3 = k
        # chunks (k idx 128c+p), cols 4-7 = q chunks
        qk_ca = np.concatenate(
            [np.concatenate([kca[rq, t].reshape(4, 128).T,
                             qca[rq, t].reshape(4, 128).T], 1)
             for t in range(3)], 1)
        # tap 3 multiplies the raw int8 matvec result, so fold the int8
        # per-row scales into it (cols 0-3 = k chunks, 4-7 = q chunks)
        s_q_c = s_qk[0:512].reshape(4, 128).T
        s_k_c = s_qk[512:1024].reshape(4, 128).T
        qk_cw = np.concatenate(
            [np.concatenate([kcw[rq, t].reshape(4, 128).T * (s_k_c if t == 3 else 1.0),
                             qcw[rq, t].reshape(4, 128).T * (s_q_c if t == 3 else 1.0)], 1)
             for t in range(4)], 1)
        # v conv in column layout [128, 8*taps]: vcol[p, 8t+cc] = v[128cc+p, t]
        v_ca = np.ascontiguousarray(
            vca[rv].reshape(8, 128, 3).transpose(1, 2, 0).reshape(128, 24))
        v_cw = vcw[rv].reshape(8, 128, 4).transpose(1, 2, 0).reshape(128, 32).copy()
        v_cw[:, 24:32] *= s_v.reshape(8, 128).T

        in_maps.append({
            "wqk_b": wqk_b, "wv_b": wv_b, "wo_b": wo_b,
            "wab": wab_sb, "state_c": st_sb,
            "hb": hb_c, "h_f32": h_f_c,
            "qkcache": np.ascontiguousarray(qk_ca),
            "qkconvw": np.ascontiguousarray(qk_cw),
            "vcache": v_ca, "vconvw": v_cw,
        })
    return in_maps


def _run(inputs, trace=False, tmpdir=None):
    _ensure_ntff_hook()
    if "nc" not in _CACHE:
        _CACHE["nc"] = _build_nc()
    nc = _CACHE["nc"]
    in_maps = _prep_in_maps(inputs)
    res = run_bass_kernel_spmd(nc, in_maps, list(range(NCORES)),
                               trace=trace, tmpdir=tmpdir)
    acc = np.zeros(H, np.float64)
    for c in range(NCORES):
        acc += res.results[c]["out"][0].astype(np.float64)
    out = acc.astype(np.float32).reshape(1, H, 1, 1)
    return out, res


def kernel(**inputs):
    out, _ = _run(inputs, trace=False)
    return out


def kernel_traced(tmpdir=None, **inputs):
    return _run(inputs, trace=True, tmpdir=tmpdir)


# revision 41
# speedup vs baseline: 1.7805x; 1.1174x over previous
"""DeltaNet decode step on 8 Trainium2 NeuronCores (tensor-parallel over heads).

Contract: kernel(**inputs) takes the FULL unsharded inputs (numpy arrays,
same keys as the reference setup_inputs()) and returns the FULL output
[1, 4096, 1, 1] float32.

Sharding (8 cores, 16 heads -> 2 heads/core):
  - Wq/Wk rows, q/k conv weights+caches: 512 rows per core
  - Wv rows, v conv weights+caches, Wo columns: 1024 per core
  - state: 2 heads per core
  - output: each core computes a partial [4096] projection; host all-reduces.

Device kernel: memory-bound mat-vec streaming. Weights are host-converted
to single bf16 and packed tile-major (each [128, 16384] SBUF tile is one
contiguous 32 KB row per partition in DRAM). wpool bufs=2 serializes the
weight DMAs into phase order (qk, v, o) so later phases don't steal HBM
bandwidth from earlier ones; the last phases stream in 1 MB quarters so
the final matmuls lag the last byte by only ~2 us.

The post-matvec chain (conv, l2norm, state update, combine) runs in
128-lane column layout. Everything that doesn't depend on v (alpha/beta
broadcast, state matvecs, alpha*ks / alpha*qs) is precomputed during the
v/o streaming phases, leaving a ~4 us serial tail: v fold -> conv tap 3
-> SiLU -> 3 wide DVE ops -> bf16 cast.
"""

import sys
import types

sys.path.insert(0, "/opt/trn_rl_repo")

import numpy as np
import ml_dtypes

import concourse.bass as bass
import concourse.mybir as mybir
import concourse.tile as tile
from concourse import bacc
from concourse.bass_utils import run_bass_kernel_spmd

BF16 = ml_dtypes.bfloat16
F32 = mybir.dt.float32
BF = mybir.dt.bfloat16
I8 = mybir.dt.int8
AF = mybir.ActivationFunctionType
OP = mybir.AluOpType

H = 4096
QK = 4096
VD = 8192
EPS = 1e-6
NCORES = 8
HPC = 2          # heads per core
RQ = 512         # q/k rows per core
RV = 1024        # v rows / Wo cols per core

_CACHE = {}


def _ensure_ntff_hook():
    """Install the axon NTFF profile hook shim (antenv.axon_hooks is absent
    in this image). Harmless if profiling is never requested."""
    if "antenv.axon_hooks" in sys.modules:
        return
    try:
        import antenv
        mod = types.ModuleType("antenv.axon_hooks")
        mod._hook = None
        mod.set_axon_ntff_profile_hook = lambda h: setattr(mod, "_hook", h)
        mod.get_axon_ntff_profile_hook = lambda: mod._hook
        sys.modules["antenv.axon_hooks"] = mod
        antenv.axon_hooks = mod
        from trn_agent_boot.trn_boot import _ntff_profile_via_ctypes
        mod._hook = _ntff_profile_via_ctypes("/opt/axon/libaxon_pjrt.so")
    except Exception:
        pass


def _build_nc():
    nc = bacc.Bacc(None)

    d = {}
    d["wqk_b"] = nc.dram_tensor("wqk_b", [256, 16384], I8, kind="ExternalInput")
    d["wv_b"] = nc.dram_tensor("wv_b", [256, 16384], I8, kind="ExternalInput")
    d["wo_b"] = nc.dram_tensor("wo_b", [256, 16384], BF, kind="ExternalInput")
    d["wab"] = nc.dram_tensor("wab", [128, 128], BF, kind="ExternalInput")
    d["state_c"] = nc.dram_tensor("state_c", [128, 2048], BF, kind="ExternalInput")
    d["hb"] = nc.dram_tensor("hb", [128, 32], BF, kind="ExternalInput")
    d["qkcache"] = nc.dram_tensor("qkcache", [128, 24], F32, kind="ExternalInput")
    d["qkconvw"] = nc.dram_tensor("qkconvw", [128, 32], F32, kind="ExternalInput")
    d["vcache"] = nc.dram_tensor("vcache", [128, 24], F32, kind="ExternalInput")
    d["vconvw"] = nc.dram_tensor("vconvw", [128, 32], F32, kind="ExternalInput")
    out_d = nc.dram_tensor("out", [1, H], BF, kind="ExternalOutput")

    with tile.TileContext(nc) as tc:
        with (
            tc.tile_pool(name="smalls", bufs=1) as sm,
            tc.tile_pool(name="wpool", bufs=4) as wp,
            tc.tile_pool(name="i8pool", bufs=2) as ip,
            tc.tile_pool(name="psum", bufs=8, space="PSUM") as pm,
        ):
            def emit():
                views = {k: d[k].rearrange("(d p) r -> d p r", p=128)
                         for k in ("wqk_b", "wv_b", "wo_b")}

                def big_tile(key, dd, cuts):
                    # 4 MB SBUF tile streamed in pieces (col offsets in cuts);
                    # finer pieces let consuming matmuls start sooner.
                    t = wp.tile([128, 16384], BF, tag="w", name="wtile")
                    for a, b in zip(cuts, cuts[1:] + [16384]):
                        nc.sync.dma_start(out=t[:, a:b],
                                          in_=views[key][dd][:, a:b])
                    return t

                def big_tile8(key, dd, cuts):
                    # int8 weights: DMA 1 B/elem, then exact on-chip cast to
                    # bf16 (int8 integers are exactly representable). Each
                    # piece's cast is split across vector/scalar/gpsimd
                    # (~237/147/153 G elem/s) so casting keeps pace with DMA.
                    t8 = ip.tile([128, 16384], I8, tag="w8", name="w8tile")
                    tb = wp.tile([128, 16384], BF, tag="w", name="wtile")
                    for a, b in zip(cuts, cuts[1:] + [16384]):
                        nc.sync.dma_start(out=t8[:, a:b],
                                          in_=views[key][dd][:, a:b])
                        c1 = a + ((b - a) * 62 // 100) // 128 * 128
                        nc.vector.tensor_copy(tb[:, a:c1], t8[:, a:c1])
                        nc.scalar.copy(tb[:, c1:b], t8[:, c1:b])
                    return tb

                H2 = [0, 8192]
                Q4 = [0, 4096, 8192, 12288]

                # Stream order qk -> v -> o. The whole q/k chain hides
                # under v/o streaming; only the short v tail (fold/conv/SiLU/
                # combine, ~3us) is exposed before the o matmuls. The o DMAs
                # are gated on v1's FIRST half so they issue while v's last
                # 2 MB streams: o bytes arrive right behind v with no bubble,
                # and the o matmuls track the o stream to the end.
                t_qk0 = big_tile8("wqk_b", 0, [0, 2048, 4096, 6144, 8192, 12288])
                t_qk1 = big_tile8("wqk_b", 1, Q4)

                # ---- small input DMAs (SWDGE keeps the HWDGE rings clear) ----
                hb = sm.tile([128, 32], BF, tag="hb")
                wab = sm.tile([128, 128], BF, tag="wab")
                st = sm.tile([128, 2048], BF, tag="st")
                qkca = sm.tile([128, 24], F32, tag="qkca")
                qkcw = sm.tile([128, 32], F32, tag="qkcw")
                vca = sm.tile([128, 24], F32, tag="vca")
                vcw = sm.tile([128, 32], F32, tag="vcw")
                for t, src in [(hb, "hb"), (wab, "wab"),
                               (qkca, "qkcache"), (qkcw, "qkconvw"),
                               (vca, "vcache"), (vcw, "vconvw"),
                               (st, "state_c")]:
                    nc.gpsimd.dma_start(out=t[:], in_=d[src][:])
                ones = sm.tile([1, 128], F32, tag="ones")
                nc.vector.memset(ones[:], 1.0)
                ones_bf = sm.tile([1, 128], BF, tag="ones_bf")
                nc.vector.memset(ones_bf[:], 1.0)
                onesc = sm.tile([128, 1], F32, tag="onesc")
                nc.vector.memset(onesc[:], 1.0)
                epst = sm.tile([1, 1], F32, tag="epst")
                nc.vector.memset(epst[:], EPS)

                # preload the ACT COPY table before Silu/Sqrt so the final
                # psum->sbuf copies don't reload it on the critical path
                cpy_warm = sm.tile([1, 1], F32, tag="cpy_warm")
                nc.scalar.copy(cpy_warm[:], epst[:])

                # ---- conv tap precompute (taps 0-2 need only cached inputs) --
                qacc_p = sm.tile([128, 8], F32, tag="qacc_p")
                qtmp = sm.tile([128, 8], F32, tag="qtmp")
                nc.vector.tensor_mul(qacc_p[:], qkca[:, 0:8], qkcw[:, 0:8])
                for tpi in (1, 2):
                    nc.vector.tensor_mul(qtmp[:], qkca[:, 8 * tpi:8 * tpi + 8],
                                         qkcw[:, 8 * tpi:8 * tpi + 8])
                    nc.vector.tensor_add(qacc_p[:], qacc_p[:], qtmp[:])
                vacc_p = sm.tile([128, 8], F32, tag="vacc_p")
                vtmp = sm.tile([128, 8], F32, tag="vtmp")
                nc.vector.tensor_mul(vacc_p[:], vca[:, 0:8], vcw[:, 0:8])
                for tpi in (1, 2):
                    nc.vector.tensor_mul(vtmp[:], vca[:, 8 * tpi:8 * tpi + 8],
                                         vcw[:, 8 * tpi:8 * tpi + 8])
                    nc.vector.tensor_add(vacc_p[:], vacc_p[:], vtmp[:])

                # ---- big streaming matvecs: one M=1 bf16 matmul per chunk ----
                def mm_piece(t, dd, ps_list, i_lo, i_hi):
                    for i in range(i_lo, i_hi):
                        cc = 16 * dd + i
                        for it, ps in enumerate(ps_list):
                            nc.tensor.matmul(
                                ps[0:1, :], hb[:, cc:cc + 1],
                                t[:, 1024 * i + 512 * it:
                                  1024 * i + 512 * it + 512],
                                start=(cc == 0), stop=(cc == 31))

                # q/k: packed [Wq | Wk]; slot 0 = q, slot 1 = k
                ps_q = pm.tile([1, 512], F32, tag="ps")
                ps_k = pm.tile([1, 512], F32, tag="ps")
                mm_piece(t_qk0, 0, [ps_q, ps_k], 0, 16)
                mm_piece(t_qk1, 1, [ps_q, ps_k], 0, 16)

                # ---- alpha/beta matvec (fp32, tiny); fills the qk->v PE gap
                ps_ab = pm.tile([1, 4], F32, tag="ps")
                for cc in range(32):
                    nc.tensor.matmul(
                        ps_ab[0:1, 0:4], hb[:, cc:cc + 1],
                        wab[:, 4 * cc:4 * cc + 4],
                        start=(cc == 0), stop=(cc == 31))
                # sigmoid(x) = silu(x)/x keeps the ACT table set to
                # {Silu, Sqrt} (2 resident slots -> no reload before v SiLU)
                ab_rec = sm.tile([1, 4], F32, tag="ab_rec")
                nc.vector.reciprocal(ab_rec[:], ps_ab[0:1, :])
                ab_sil = sm.tile([1, 4], F32, tag="ab_sil")
                nc.scalar.activation(ab_sil[:], ps_ab[:], AF.Silu)
                ab = sm.tile([1, 4], F32, tag="ab")
                nc.vector.tensor_mul(ab[:], ab_sil[:], ab_rec[:])

                qrow = sm.tile([1, 512], BF, tag="qrow")
                nc.vector.tensor_copy(qrow[:], ps_q[0:1, :])
                krow = sm.tile([1, 512], BF, tag="krow")
                nc.vector.tensor_copy(krow[:], ps_k[0:1, :])

                # q/k chain tiles, 128-lane column layout (cols 0-3 = k
                # chunks, 4-7 = q chunks); PE pieces injected into the v
                # streaming phase to keep the PE warm through DMA waits.
                t_qk = pm.tile([128, 8], F32, tag="ps")
                qkcol = sm.tile([128, 8], F32, tag="qkcol")
                qacc = sm.tile([128, 8], F32, tag="qacc")
                x1 = sm.tile([128, 8], F32, tag="x1")
                sq = sm.tile([128, 8], F32, tag="sq")
                ps_ss = pm.tile([1, 8], F32, tag="ps")
                ssr = sm.tile([1, 8], F32, tag="ssr")
                ssh = sm.tile([1, 4], F32, tag="ssh")
                srt = sm.tile([1, 4], F32, tag="srt")
                rin = sm.tile([1, 4], F32, tag="rin")
                t_rn = pm.tile([128, 4], F32, tag="ps")
                rbc = sm.tile([128, 4], F32, tag="rbc")
                qkn = sm.tile([128, 8], F32, tag="qkn")
                qkn_bf = sm.tile([128, 8], BF, tag="qkn_bf")
                dm = sm.tile([128, 4], F32, tag="dm")
                ps_dot = pm.tile([1, 4], F32, tag="ps")
                dotr = sm.tile([1, 4], F32, tag="dotr")
                dot = sm.tile([1, 2], F32, tag="dot")
                bd = sm.tile([1, 2], F32, tag="bd")
                abrow = sm.tile([1, 16], F32, tag="abrow")
                t_bc = pm.tile([128, 16], F32, tag="ps")
                abc = sm.tile([128, 16], F32, tag="abc")
                ps_stc = pm.tile([128, 16], F32, tag="ps")
                ksS = sm.tile([128, 8], F32, tag="ksS")
                qsS = sm.tile([128, 8], F32, tag="qsS")

                def chain_pe_0():
                    # raw q/k rows -> columns (K=1 outer products)
                    for c in range(4):
                        nc.tensor.matmul(t_qk[:, c:c + 1],
                                         krow[0:1, 128 * c:128 * c + 128],
                                         ones_bf[0:1, 0:1], start=True, stop=True)
                        nc.tensor.matmul(t_qk[:, 4 + c:5 + c],
                                         qrow[0:1, 128 * c:128 * c + 128],
                                         ones_bf[0:1, 0:1], start=True, stop=True)
                    nc.vector.tensor_copy(qkcol[:], t_qk[:])
                    # conv tap 3 + SiLU in columns
                    nc.vector.tensor_mul(qacc[:], qkcol[:], qkcw[:, 24:32])
                    nc.vector.tensor_add(qacc[:], qacc_p[:], qacc[:])
                    nc.scalar.activation(x1[:], qacc[:], AF.Silu)
                    nc.vector.tensor_mul(sq[:], x1[:], x1[:])

                def chain_pe_1():
                    # per-column sum of squares, then per-head 1/sqrt
                    nc.tensor.matmul(ps_ss[0:1, :], onesc[:, 0:1], sq[:],
                                     start=True, stop=True)
                    nc.vector.tensor_copy(ssr[:], ps_ss[0:1, :])
                    nc.vector.reduce_sum(
                        ssh[0:1, 0:4],
                        ssr[0:1, :].rearrange("a (g t) -> a g t", t=2),
                        axis=mybir.AxisListType.X)
                    nc.scalar.activation(srt[:], ssh[:], AF.Sqrt,
                                         bias=epst[0:1, 0:1])
                    nc.vector.reciprocal(rin[:], srt[:])

                def chain_pe_2():
                    # broadcast 1/norm, normalize columns
                    for j in range(4):
                        nc.tensor.matmul(t_rn[:, j:j + 1], ones[0:1, :],
                                         rin[0:1, j:j + 1], start=True, stop=True)
                    nc.vector.tensor_copy(rbc[:], t_rn[:])
                    for g in range(4):  # k_h0, k_h1, q_h0, q_h1 col pairs
                        nc.vector.tensor_scalar(
                            out=qkn[:, 2 * g:2 * g + 2],
                            in0=x1[:, 2 * g:2 * g + 2],
                            scalar1=rbc[:, g:g + 1], scalar2=None, op0=OP.mult)
                    nc.vector.tensor_copy(qkn_bf[:], qkn[:])
                    # q.k dot per head
                    nc.vector.tensor_mul(dm[:], qkn[:, 4:8], qkn[:, 0:4])
                    nc.tensor.matmul(ps_dot[0:1, :], onesc[:, 0:1], dm[:],
                                     start=True, stop=True)
                    nc.vector.tensor_copy(dotr[:], ps_dot[0:1, :])
                    nc.vector.reduce_sum(
                        dot[0:1, 0:2],
                        dotr[0:1, :].rearrange("a (g t) -> a g t", t=2),
                        axis=mybir.AxisListType.X)
                    nc.vector.tensor_mul(bd[:], ab[0:1, 2:4], dot[0:1, 0:2])
                    # abrow = (a0 a0 a0 a0 a1 a1 a1 a1 | bd0 x4 bd1 x4)
                    for hh in range(HPC):
                        nc.vector.tensor_scalar(
                            out=abrow[0:1, 4 * hh:4 * hh + 4],
                            in0=ones[0:1, 0:4],
                            scalar1=ab[0:1, hh:hh + 1], scalar2=None, op0=OP.mult)
                        nc.vector.tensor_scalar(
                            out=abrow[0:1, 8 + 4 * hh:12 + 4 * hh],
                            in0=ones[0:1, 0:4],
                            scalar1=bd[0:1, hh:hh + 1], scalar2=None, op0=OP.mult)
                    nc.tensor.matmul(t_bc[:, 0:16], ones[0:1, 0:128],
                                     abrow[0:1, 0:16], start=True, stop=True)
                    nc.vector.tensor_copy(abc[:], t_bc[:])

                def chain_pe_3():
                    # state matvecs (bf16 state x bf16 normalized q/k)
                    for hh in range(HPC):
                        for which in range(2):  # 0 -> k, 1 -> q
                            for vc in range(4):
                                col = 8 * which + 4 * hh + vc
                                for d2 in range(2):
                                    blk = 2 * hh + d2
                                    nc.tensor.matmul(
                                        ps_stc[:, col:col + 1],
                                        st[:, 512 * blk + 128 * vc:
                                           512 * blk + 128 * vc + 128],
                                        qkn_bf[:, 4 * which + 2 * hh + d2:
                                               4 * which + 2 * hh + d2 + 1],
                                        start=(d2 == 0), stop=(d2 == 1))
                    # alpha-scaled retrieved/out state (combine precompute)
                    nc.vector.tensor_mul(ksS[:], ps_stc[:, 0:8], abc[:, 0:8])
                    nc.vector.tensor_mul(qsS[:], ps_stc[:, 8:16], abc[:, 0:8])

                # ---- v matvec; chain PE work injected between its pieces ----
                ps_v0 = pm.tile([1, 512], F32, tag="ps")
                ps_v1 = pm.tile([1, 512], F32, tag="ps")
                t_v0 = big_tile8("wv_b", 0, Q4)
                mm_piece(t_v0, 0, [ps_v0, ps_v1], 0, 8)
                chain_pe_0()
                mm_piece(t_v0, 0, [ps_v0, ps_v1], 8, 16)
                chain_pe_1()
                # v1 streams in 1 MB pieces: at the v/o boundary it holds
                # most of the in-flight DMA slots, so round-robin service
                # gives v's tail most of the bandwidth without any gate.
                t_v1 = big_tile8("wv_b", 1, Q4)

                mm_piece(t_v1, 1, [ps_v0, ps_v1], 0, 4)
                chain_pe_2()
                mm_piece(t_v1, 1, [ps_v0, ps_v1], 4, 8)
                chain_pe_3()
                mm_piece(t_v1, 1, [ps_v0, ps_v1], 8, 12)
                mm_piece(t_v1, 1, [ps_v0, ps_v1], 12, 16)

                # ---- v tail: rows -> bf16 -> columns -> conv -> SiLU ----
                vsb = sm.tile([1, 1024], BF, tag="vsb")
                nc.vector.tensor_copy(vsb[0:1, 0:512], ps_v0[0:1, :])
                nc.vector.tensor_copy(vsb[0:1, 512:1024], ps_v1[0:1, :])
                t_v = pm.tile([128, 8], F32, tag="ps")
                for j in range(8):
                    nc.tensor.matmul(t_v[:, j:j + 1],
                                     vsb[0:1, 128 * j:128 * j + 128],
                                     ones_bf[0:1, 0:1], start=True, stop=True)
                vcol = sm.tile([128, 8], F32, tag="vcol")
                nc.vector.tensor_copy(vcol[:], t_v[:])
                vacc = sm.tile([128, 8], F32, tag="vacc")
                nc.vector.tensor_mul(vacc[:], vcol[:], vcw[:, 24:32])
                nc.vector.tensor_add(vacc[:], vacc_p[:], vacc[:])
                v1c = sm.tile([128, 8], F32, tag="v1c")
                nc.scalar.activation(v1c[:], vacc[:], AF.Silu)

                # ---- combine: ov = a*qs + (b*dot)*(v1 - a*ks) ----
                ovc = sm.tile([128, 8], F32, tag="ovc")
                err8 = sm.tile([128, 8], F32, tag="err8")
                nc.vector.tensor_sub(err8[:], v1c[:], ksS[:])
                nc.vector.tensor_mul(err8[:], err8[:], abc[:, 8:16])
                nc.vector.tensor_add(ovc[:], qsS[:], err8[:])
                ov_hi = sm.tile([128, 8], BF, tag="ov_hi")
                nc.vector.tensor_copy(ov_hi[:], ovc[:])

                # ---- output projection ----
                ps_o = [pm.tile([1, 512], F32, tag="ps", name=f"ps_o{i}")
                        for i in range(8)]
                out_sb = sm.tile([1, H], BF, tag="out_sb")
                for dd in range(2):
                    t = big_tile("wo_b", dd, Q4 if dd == 0 else [0, 8192, 12288, 14336])
                    for i in range(4):
                        j = 4 * dd + i
                        for it in range(8):
                            sl = slice(4096 * i + 512 * it,
                                       4096 * i + 512 * it + 512)
                            nc.tensor.matmul(ps_o[it][0:1, :], ov_hi[:, j:j + 1],
                                             t[:, sl], start=(j == 0),
                                             stop=(j == 7))
                for half in range(2):
                    for it in range(4 * half, 4 * half + 4):
                        dst = out_sb[0:1, 512 * it:512 * it + 512]
                        if it % 2 == 0:
                            nc.vector.tensor_copy(dst, ps_o[it][0:1, :])
                        else:
                            nc.scalar.copy(dst, ps_o[it][0:1, :])
                    sl = slice(2048 * half, 2048 * half + 2048)
                    nc.sync.dma_start(out=out_d[0:1, sl], in_=out_sb[0:1, sl])

            emit()

    nc.finalize()
    return nc


def _tile_major(mT):
    """mT [R, C] bf16 -> packed so each 128-row device tile is one
    contiguous 32 KB span per partition. ni = 16384//C."""
    R, C = mT.shape
    ni = 16384 // C
    dd = R // (128 * ni)
    return np.ascontiguousarray(
        mT.reshape(dd, ni, 128, C).transpose(0, 2, 1, 3).reshape(dd * 128, ni * C))


def _prep_in_maps(inputs):
    f32 = np.float32
    hid = np.asarray(inputs["hidden_states"], f32)[0, :, 0, 0]     # [4096]
    Wq = np.asarray(inputs["Wq"], f32)
    Wk = np.asarray(inputs["Wk"], f32)
    Wv = np.asarray(inputs["Wv"], f32)
    Wo = np.asarray(inputs["Wo"], f32)
    Wa = np.asarray(inputs["Wa"], f32)
    Wb = np.asarray(inputs["Wb"], f32)
    qcw = np.asarray(inputs["q_conv_w"], f32)[0]                   # [QK, 4]
    kcw = np.asarray(inputs["k_conv_w"], f32)[0]
    vcw = np.asarray(inputs["v_conv_w"], f32)[0]                   # [VD, 4]
    qca = np.asarray(inputs["q_cache"], f32)[0]                    # [QK, 3]
    kca = np.asarray(inputs["k_cache"], f32)[0]
    vca = np.asarray(inputs["v_cache"], f32)[0]                    # [VD, 3]
    state = np.asarray(inputs["state"], f32)[0]                    # [16,256,512]

    cols = lambda v: np.ascontiguousarray(v.reshape(32, 128).T)
    hb_c = cols(hid.astype(BF16))

    in_maps = []
    for c in range(NCORES):
        rq = slice(c * RQ, (c + 1) * RQ)
        rv = slice(c * RV, (c + 1) * RV)
        # packed [Wq ; Wk] rows -> int8 with per-row scales; the scales
        # ride in the tap-3 conv weights (q1 = sum caches*w + (s*q_raw)*w3)
        wqk = np.concatenate([Wq[rq], Wk[rq]], axis=0)             # [1024, 4096]
        s_qk = np.abs(wqk).max(axis=1) / 127.0                     # [1024]
        wqk8 = np.round(wqk / s_qk[:, None]).astype(np.int8)
        wqk_b = _tile_major(np.ascontiguousarray(wqk8.T))
        wv = Wv[rv]                                                # [1024, 4096]
        s_v = np.abs(wv).max(axis=1) / 127.0
        wv8 = np.round(wv / s_v[:, None]).astype(np.int8)
        wv_b = _tile_major(np.ascontiguousarray(wv8.T))
        wo_b = _tile_major(np.ascontiguousarray(Wo[:, rv]).astype(BF16).T.copy())

        wab = np.concatenate([Wa[2 * c:2 * c + 2], Wb[2 * c:2 * c + 2]], 0)
        wab_sb = np.ascontiguousarray(
            wab.reshape(4, 32, 128).transpose(2, 1, 0).reshape(128, 128)).astype(BF16)
        st_sb = np.ascontiguousarray(
            state[2 * c:2 * c + 2].reshape(2, 2, 128, 512)
            .transpose(2, 0, 1, 3).reshape(128, 2048)).astype(BF16)

        # q/k conv in column layout [128, 8*taps]: per tap, cols 0-3 = k
        # chunks (k idx 128c+p), cols 4-7 = q chunks
        qk_ca = np.concatenate(
            [np.concatenate([kca[rq, t].reshape(4, 128).T,
                             qca[rq, t].reshape(4, 128).T], 1)
             for t in range(3)], 1)
        # tap 3 multiplies the raw int8 matvec result, so fold the int8
        # per-row scales into it (cols 0-3 = k chunks, 4-7 = q chunks)
        s_q_c = s_qk[0:512].reshape(4, 128).T
        s_k_c = s_qk[512:1024].reshape(4, 128).T
        qk_cw = np.concatenate(
            [np.concatenate([kcw[rq, t].reshape(4, 128).T * (s_k_c if t == 3 else 1.0),
                             qcw[rq, t].reshape(4, 128).T * (s_q_c if t == 3 else 1.0)], 1)
             for t in range(4)], 1)
        # v conv in column layout [128, 8*taps]: vcol[p, 8t+cc] = v[128cc+p, t]
        v_ca = np.ascontiguousarray(
            vca[rv].reshape(8, 128, 3).transpose(1, 2, 0).reshape(128, 24))
        v_cw = vcw[rv].reshape(8, 128, 4).transpose(1, 2, 0).reshape(128, 32).copy()
        v_cw[:, 24:32] *= s_v.reshape(8, 128).T

        in_maps.append({
            "wqk_b": wqk_b, "wv_b": wv_b, "wo_b": wo_b,
            "wab": wab_sb, "state_c": st_sb,
            "hb": hb_c,
            "qkcache": np.ascontiguousarray(qk_ca),
            "qkconvw": np.ascontiguousarray(qk_cw),
            "vcache": v_ca, "vconvw": v_cw,
        })
    return in_maps


def _run(inputs, trace=False, tmpdir=None):
    _ensure_ntff_hook()
    if "nc" not in _CACHE:
        _CACHE["nc"] = _build_nc()
    nc = _CACHE["nc"]
    in_maps = _prep_in_maps(inputs)
    res = run_bass_kernel_spmd(nc, in_maps, list(range(NCORES)),
                               trace=trace, tmpdir=tmpdir)
    acc = np.zeros(H, np.float64)
    for c in range(NCORES):
        acc += res.results[c]["out"][0].astype(np.float64)
    out = acc.astype(np.float32).reshape(1, H, 1, 1)
    return out, res


def kernel(**inputs):
    out, _ = _run(inputs, trace=False)
    return out


def kernel_traced(tmpdir=None, **inputs):
    return _run(inputs, trace=True, tmpdir=tmpdir)
